# revision 23
# baseline (speedup 1.0000x reference)
"""LSTMCell on 8 Trainium2 NeuronCores, data-parallel over the batch.

Full inputs: x/h_t/c_t [65536,128] f32, 8 gate weight matrices [128,128],
4 biases [128]. Returns (h_new, c_new) as [65536,128] f32 each.

Design (v14): the v13 pipeline was ACT-bound: 5 transcendental columns
per batch element (4 gate sigmoids + tanh(c')) at the ACT engine's
fixed (N+352)/1.2ns => ~40us of ACT busy.  v14 moves tanh(c') OFF the
ACT engine into two custom fused DVE ops (injected per-NEFF via the
dve_ops extension point):
    LSTM_TANH_RECIP: r = 1/(c'^2+D) via BITWISE_NOT exponent-flip seed
        + one inline Newton step (fp32 internally, fp16 in/out)
    LSTM_TANH_ZMUL:  z = clamp((r*c2 + c1)*c', -1, 1) ~= tanh(c')
(minimax-fitted c1,c2,D: max |z - tanh| ~= 3.7e-3 over the fp16 path).
ACT now does ONLY the 4-bank sigmoid quad per group: 16 x 2000ns = 32us.

Engine balance per pair of 512-col groups (1024 batch cols):
  ACT  sigmoid quad [128,2048] x2           ~4.0us  <- pacer
  PE   16 fp16 matmuls (weights stationary) ~3.5us
  Pool u' = (sg-0.5)*si ; fc = sf*c         ~3.6us  (was idle)
  DVE  c' = 2u'+fc ; r ; z ; h' = z*so      ~3.8us
  DMA  in 0.75MB + out 0.5MB                ~3.6us
Gate order in quad/PSUM: [o | i | f | 2g] (g prescaled by 2 on host for
the tanh-via-sigmoid trick, fp16 operands as v13: bf16 rounding was the
dominant error term).

Fill/tail: x/h stream on the sync queue in consumption order; c stream
+ weights on the gpsimd queue (big c chunks issued between pool ops so
they can't steal DMA bandwidth from the x/h stream the PE waits on);
9 junk warmup matmuls keep the PE HAM activity window alive - fewer
leaves the PE clock-gated at half rate for the WHOLE kernel.  Last pair
runs its whole chain per-group on the DVE to shorten the kernel tail.
"""
import numpy as np
import ml_dtypes
from contextlib import ExitStack

import concourse.bass as bass
import concourse.tile as tile
from concourse import bacc, mybir
from concourse.bass_utils import run_bass_kernel_spmd

from concourse import dve_ops as _dop
from concourse.dve_spec import (
    Spec, Src0, Src1, C0, C1, C2, Bin, AluOp as DAlu, maxx, minn, sq,
    lower as _dve_lower, _has_src1,
)
from concourse.dve_uop import DveOpSpec

F32 = mybir.dt.float32
F16 = mybir.dt.float16
BF16 = mybir.dt.bfloat16
NPBF = ml_dtypes.bfloat16
AF = mybir.ActivationFunctionType
ALU = mybir.AluOpType

NCORES = 8
BC = 8192            # batch rows per core
GW = 512             # batch columns per group (one PSUM bank)
NG = BC // GW        # 16 groups
H = 128              # hidden size
# x/h chunks in groups (sync queue): small first for fast fill
ICHUNKS = [(0, 1), (1, 1), (2, 2), (4, 4), (8, 8)]
# c chunks in groups (gpsimd queue, pair-aligned)
CCHUNKS = [(0, 2), (2, 2), (4, 4), (8, 4), (12, 4)]
# output chunks (start group, n groups)
OCHUNKS = [(0, 8), (8, 4), (12, 2), (14, 2)]
# HAM un-throttle needs ~3.4us of SUSTAINED PE activity; at ~427ns cold
# issue per N=512 matmul that is >=8 warmups. Fewer warmups leave the PE
# at half clock for the WHOLE kernel (measured: 75us vs 58us).
NWARM = 9
# pairs whose tanh(c') runs on ACT instead of the custom DVE path
ACT_TANH_PAIRS = (1, 4)

# --- custom fused DVE ops (rational tanh) ---------------------------------
# recip seed/NR consts (same derivation as RECIPROCAL_APPROX_FAST's y1)
TR0, TR1 = -0.23549792, 2.0017324
# minimax fit of clamp(x*(TC1 + TC2/(x^2+TD))) ~= tanh(x) over the fp16
# path (joint fit including the approximate reciprocal): max err 3.7e-3
TC1, TC2, TD = 0.12762096, 2.40399202, 2.78807243


def _register_dve_op(name, spec, subdim=False):
    if name in _dop._SUB_OPCODE_FOR_NAME:
        return next(op for op in _dop.OPS if op.name == name)
    row = _dop._CUSTOM_DVE_ROW_BASE + len(_dop.OPS)
    assert row < 0x20, "custom DVE row overflow"
    shas = {}
    for ver in ("v3", "v4"):
        try:
            tmp = DveOpSpec(name=name, opcode=row,
                            uops=_dve_lower(spec, ver=ver),
                            rd1_en=_has_src1(spec))
            shas[ver] = tmp.sha(ver)
        except Exception:
            pass
    op = _dop.DveOp(name, spec, subdim, shas)
    _dop.OPS.append(op)
    _dop._SUB_OPCODE_FOR_NAME[name] = row
    _dop.CUSTOM_DVE_SPECS[name] = spec
    return op


def _ref_tanh_recip(in0, in1, c0, c1, c2):
    d = (in0.astype(np.float32) ** 2 + np.float32(c0)).astype(np.float32)
    nd = (~d.view(np.int32)).view(np.float32)
    y0 = nd * np.float32(c1)
    return y0 * (np.float32(c2) - d * y0)


def _ref_tanh_zmul(in0, in1, c0, c1, c2):
    t = ((in0.astype(np.float32) * np.float32(c0) + np.float32(c1))
         * in1.astype(np.float32))
    return np.maximum(np.minimum(t, np.float32(c2)), -np.float32(c2))


_d = sq(Src0) + C0
_nd = Bin(DAlu.BITWISE_NOT, _d, _d)
_y0 = _nd * C1
TANH_RECIP = _register_dve_op(
    "LSTM_TANH_RECIP", Spec(body=_y0 * (C2 - _d * _y0),
                            reference=_ref_tanh_recip))

_t = (Src0 * C0 + C1) * Src1
TANH_ZMUL = _register_dve_op(
    "LSTM_TANH_ZMUL", Spec(body=maxx(minn(_t, C2), -C2),
                           reference=_ref_tanh_zmul))

# clamp-free (Src0*c0 + c1)*Src1 - no imm2, so 3D (2-free-dim) operands
# are allowed (STT struct). Used for ig = (2*sg - 1)*si.
AFFINE_MUL = _register_dve_op(
    "LSTM_AFFINE_MUL",
    Spec(body=_t, reference=lambda in0, in1, c0, c1, c2:
         (in0.astype(np.float32) * np.float32(c0) + np.float32(c1))
         * in1.astype(np.float32)))

_CACHE = {}


def _build(has_bias: bool):
    nc = bacc.Bacc("TRN2", target_bir_lowering=False, debug=False)
    xt = nc.dram_tensor("xt", [H, BC], F16, kind="ExternalInput").ap()
    ht = nc.dram_tensor("ht", [H, BC], F16, kind="ExternalInput").ap()
    ct = nc.dram_tensor("ct", [H, BC], F16, kind="ExternalInput").ap()
    wxt = nc.dram_tensor("wxt", [H, 4 * H], F16, kind="ExternalInput").ap()
    wht = nc.dram_tensor("wht", [H, 4 * H], F16, kind="ExternalInput").ap()
    if has_bias:
        bias = nc.dram_tensor("bias", [H, 4], F32, kind="ExternalInput").ap()
    hnt = nc.dram_tensor("hnt", [H, BC], F16, kind="ExternalOutput").ap()
    cnt = nc.dram_tensor("cnt", [H, BC], F16, kind="ExternalOutput").ap()

    with tile.TileContext(nc) as tc:
        with ExitStack() as ctx:
            const = ctx.enter_context(tc.tile_pool(name="const", bufs=1))
            ina = ctx.enter_context(tc.tile_pool(name="ina", bufs=1))
            qp = ctx.enter_context(tc.tile_pool(name="qp", bufs=2, space="PSUM"))
            tp = ctx.enter_context(tc.tile_pool(name="tp", bufs=3))
            sp = ctx.enter_context(tc.tile_pool(name="sp", bufs=7))
            op = ctx.enter_context(tc.tile_pool(name="op", bufs=3))

            xts, hts = [], []
            for ci, (cs, cw) in enumerate(ICHUNKS):
                xts.append(ina.tile([H, cw * GW], F16, name=f"x{ci}"))
                hts.append(ina.tile([H, cw * GW], F16, name=f"h{ci}"))
            cts = [ina.tile([H, cw * GW], F16, name=f"c{ci}")
                   for ci, (cs, cw) in enumerate(CCHUNKS)]

            # gpsimd queue first: warmup/dummy memsets (so PE warmups are
            # not blocked behind DMA issues), then weights, then the two
            # small leading c chunks.  The big c chunks are issued later,
            # between pool ops, so their transfers can't crowd out the
            # x/h stream during the fill.  NOTE: never issue DMA from the
            # scalar queue - HWDGE on the Activation engine evicts its
            # ACT table (forces a ~1.3us reload).
            junk = const.tile([H, GW], F16)
            nc.gpsimd.memset(junk[:], 0.0)
            dummy = const.tile([H, 8], F32)
            nc.gpsimd.memset(dummy[:], 0.0)
            wx_sb = const.tile([H, 4 * H], F16)
            nc.gpsimd.dma_start(wx_sb[:], wxt)
            wh_sb = const.tile([H, 4 * H], F16)
            nc.gpsimd.dma_start(wh_sb[:], wht)
            if has_bias:
                b_sb = const.tile([H, 4], F32)
                nc.gpsimd.dma_start(b_sb[:], bias)

            def cstart(ci, eng=None):
                cs, cw = CCHUNKS[ci]
                (eng or nc.gpsimd).dma_start(
                    cts[ci][:], ct[:, cs * GW:(cs + cw) * GW])

            # sync queue: x/h/c streams interleaved in consumption order
            # (single queue => transfer order matches consumption order)
            for ci, (cs, cw) in enumerate(ICHUNKS):
                nc.sync.dma_start(xts[ci][:], xt[:, cs * GW:(cs + cw) * GW])
                nc.sync.dma_start(hts[ci][:], ht[:, cs * GW:(cs + cw) * GW])
                if ci == 0:
                    cstart(0, nc.sync)
                elif ci == 1:
                    cstart(1, nc.sync)

            # ACT table preload (sigmoid) overlaps the DMA fill
            dummy2 = const.tile([H, 8], F32)
            nc.scalar.activation(dummy2[:], dummy[:], AF.Sigmoid)

            def in_slice(tiles, chunks, g, w):
                c0 = g * GW
                for ci, (cs, cw) in enumerate(chunks):
                    if c0 >= cs * GW and c0 + w <= (cs + cw) * GW:
                        return tiles[ci][:, c0 - cs * GW:c0 - cs * GW + w]
                raise AssertionError("slice straddles input chunks")

            # HAM warmup on the junk tile while DMAs stream
            warm = qp.tile([H, 2048], F32, name="warm", tag="quad")
            for _ in range(NWARM):
                nc.tensor.matmul(warm[:, 0:GW], junk[:, 0:H], junk[:],
                                 start=True, stop=True)

            NP = NG // 2  # pairs

            # pair -> (chunk_start_group, chunk_width, local_offset, is_last)
            pair_chunk = {}
            for cs, cw in OCHUNKS:
                for g in range(cs, cs + cw, 2):
                    pair_chunk[g // 2] = (cs, cw * GW, (g - cs) * GW,
                                          g + 2 == cs + cw)

            cn_hn = {}
            cn_buf = hn_buf = None
            sig2s = {}
            pend_h = None
            for P in range(NP):
                g0 = 2 * P
                cs, cw, lo, last = pair_chunk[P]
                if lo == 0:
                    cn_buf = op.tile([H, cw], F16, name=f"cn{g0}", tag="cn")
                    hn_buf = op.tile([H, cw], F16, name=f"hn{g0}", tag="hn")
                cn_hn[P] = (cn_buf, hn_buf)
                sig2 = sp.tile([H, 4096], BF16, name=f"s{P}", tag="sig")
                sig2s[P] = sig2
                lastP = P == NP - 1

                for gg in range(2):
                    g = g0 + gg
                    xs = in_slice(xts, ICHUNKS, g, GW)
                    hs = in_slice(hts, ICHUNKS, g, GW)
                    split = (lastP or P == 0) and not has_bias
                    quad = qp.tile([H, 2048], F32, name=f"q{g}", tag="quad")
                    # sigmoid output is PERMUTED to bank-major pair layout:
                    # sig2 = [o_g0 o_g1 | i_g0 i_g1 | f_g0 f_g1 | g_g0 g_g1]
                    # so every downstream chain op gets 2D contiguous
                    # [H,1024] operands (3D APs measured ~1.5x slower).
                    so = sig2[:].rearrange("p (b x) -> p b x",
                                           b=4)[:, :, gg * GW:(gg + 1) * GW]
                    q4 = quad[:].rearrange("p (b x) -> p b x", b=4)
                    for k in ([1, 2, 3, 0] if split else range(4)):
                        nc.tensor.matmul(quad[:, k * GW:(k + 1) * GW],
                                         wx_sb[:, k * H:(k + 1) * H], xs,
                                         start=True, stop=False)
                        nc.tensor.matmul(quad[:, k * GW:(k + 1) * GW],
                                         wh_sb[:, k * H:(k + 1) * H], hs,
                                         start=False, stop=True)
                    if has_bias:
                        for k in range(4):
                            nc.scalar.activation(
                                so[:, k:k + 1, :], q4[:, k:k + 1, :],
                                AF.Sigmoid, bias=b_sb[:, k:k + 1])
                    elif split:
                        # i/f/2g banks first: unblocks the pool/DVE chain;
                        # the o bank (only needed by h') trails
                        nc.scalar.activation(so[:, 1:4, :], q4[:, 1:4, :],
                                             AF.Sigmoid)
                        nc.scalar.activation(so[:, 0:1, :], q4[:, 0:1, :],
                                             AF.Sigmoid)
                    else:
                        nc.scalar.activation(so, q4, AF.Sigmoid)

                def sl(bank, gg=None):
                    if gg is None:
                        return sig2[:, bank * 2 * GW:(bank + 1) * 2 * GW]
                    o = bank * 2 * GW + gg * GW
                    return sig2[:, o:o + GW]

                if lastP:
                    if pend_h is not None:
                        pend_h[0]()
                        pend_h[1]()
                        pend_h = None
                    # tail: whole chain per-group on the DVE (pool's slower
                    # ops would sit on the critical path) + per-group DMAs
                    for gg in range(2):
                        g = g0 + gg
                        lg = lo + gg * GW
                        cps = cn_buf[:, lg:lg + GW]
                        ig = tp.tile([H, GW], F16, name=f"ig{g}", tag="ig")
                        nc.vector._custom_dve(
                            AFFINE_MUL, out=ig[:], in0=sl(3, gg),
                            in1=sl(1, gg), s0=2.0, s1=-1.0)
                        fc = tp.tile([H, GW], F16, name=f"fc{g}", tag="fc")
                        nc.vector.tensor_mul(
                            fc[:], sl(2, gg),
                            in_slice(cts, CCHUNKS, g, GW))
                        nc.vector.tensor_add(cps, ig[:], fc[:])
                        nc.sync.dma_start(
                            cnt[:, (cs + gg * (cw // GW - 1)) * GW:
                                (cs + gg * (cw // GW - 1)) * GW + GW], cps)
                        rt = tp.tile([H, GW], F16, name=f"r{g}", tag="rt")
                        nc.vector._custom_dve(TANH_RECIP, out=rt[:], in0=cps,
                                              s0=TD, s1=TR0, imm2=TR1)
                        zt = tp.tile([H, GW], F16, name=f"z{g}", tag="zt")
                        nc.vector._custom_dve(TANH_ZMUL, out=zt[:], in0=rt[:],
                                              in1=cps, s0=TC2, s1=TC1,
                                              imm2=1.0)
                        nc.vector.tensor_mul(hn_buf[:, lg:lg + GW],
                                             sl(0, gg), zt[:])
                        nc.sync.dma_start(
                            hnt[:, (cs + gg * (cw // GW - 1)) * GW:
                                (cs + gg * (cw // GW - 1)) * GW + GW],
                            hn_buf[:, lg:lg + GW])
                    continue

                W2 = 2 * GW
                cpr = cn_buf[:, lo:lo + W2]
                # deferred ACT-tanh work from one pair back: its tanh rides
                # the ACT queue right after this pair's sigmoids
                if pend_h is not None:
                    pend_h[0]()
                # pool: fc = sf * c (plain tensor_tensor - the only op kind
                # walrus accepts on Pool; ONE [H,1024] op per pair is all
                # the slow Q7 cores can sustain at the 4us/pair sigma pace)
                fc = tp.tile([H, W2], F16, name=f"fc{P}", tag="fc")
                nc.gpsimd.tensor_mul(fc[:], sl(2),
                                     in_slice(cts, CCHUNKS, g0, W2))
                # DVE: ig = (2*sg - 1) * si (fused affine-mul custom op)
                ig = tp.tile([H, W2], F16, name=f"ig{P}", tag="ig")
                nc.vector._custom_dve(AFFINE_MUL, out=ig[:], in0=sl(3),
                                      in1=sl(1), s0=2.0, s1=-1.0)
                # big c chunks issued from the pool queue, paced by compute
                if P == 0:
                    cstart(2)
                    cstart(3)
                elif P == 1:
                    cstart(4)
                nc.vector.tensor_add(cpr, ig[:], fc[:])
                if last:
                    nc.sync.dma_start(cnt[:, cs * GW:cs * GW + cw], cn_buf[:])
                if pend_h is not None:
                    pend_h[1]()
                    pend_h = None
                if P in ACT_TANH_PAIRS:
                    # tanh(c') on ACT for this pair (it has ~1.15us/pair of
                    # slack vs the DVE's custom path; blending equalizes
                    # ACT and DVE at ~4.4us/pair)
                    def mk_pend(sig2=sig2, cpr=cpr, hn_buf=hn_buf, lo=lo,
                                cs=cs, cw=cw, last=last, P=P):
                        tc_t = tp.tile([H, W2], BF16, name=f"tc{P}",
                                       tag="zt")
                        def emit_tanh():
                            nc.scalar.activation(tc_t[:], cpr, AF.Tanh)
                        def emit_h():
                            nc.vector.tensor_mul(hn_buf[:, lo:lo + W2],
                                                 sig2[:, 0:W2], tc_t[:])
                            if last:
                                nc.sync.dma_start(
                                    hnt[:, cs * GW:cs * GW + cw], hn_buf[:])
                        return (emit_tanh, emit_h)
                    pend_h = mk_pend()
                else:
                    rt = tp.tile([H, W2], F16, name=f"r{P}", tag="rt")
                    nc.vector._custom_dve(TANH_RECIP, out=rt[:], in0=cpr,
                                          s0=TD, s1=TR0, imm2=TR1)
                    zt = tp.tile([H, W2], F16, name=f"z{P}", tag="zt")
                    nc.vector._custom_dve(TANH_ZMUL, out=zt[:], in0=rt[:],
                                          in1=cpr, s0=TC2, s1=TC1, imm2=1.0)
                    nc.vector.tensor_mul(hn_buf[:, lo:lo + W2],
                                         sig2[:, 0:W2], zt[:])
                    if last:
                        nc.sync.dma_start(hnt[:, cs * GW:cs * GW + cw],
                                          hn_buf[:])
    nc.compile()
    return nc


def _run(inputs, trace=False, tmpdir=None):
    x = np.asarray(inputs["x"], dtype=np.float32)
    h = np.asarray(inputs["h_t"], dtype=np.float32)
    c = np.asarray(inputs["c_t"], dtype=np.float32)
    # gate order [o, i, f, g]; W_g/b_g scaled by 2 for the tanh-via-sigmoid
    wx = np.concatenate([inputs["W_io"], inputs["W_ii"], inputs["W_if"],
                         2.0 * np.asarray(inputs["W_ig"])], axis=0)
    wh = np.concatenate([inputs["W_ho"], inputs["W_hi"], inputs["W_hf"],
                         2.0 * np.asarray(inputs["W_hg"])], axis=0)
    b = np.concatenate([inputs["b_o"], inputs["b_i"], inputs["b_f"],
                        2.0 * np.asarray(inputs["b_g"])], axis=0)
    wxt = np.ascontiguousarray(wx.T).astype(np.float16)
    wht = np.ascontiguousarray(wh.T).astype(np.float16)
    has_bias = bool(np.any(b))

    key = has_bias
    if key not in _CACHE:
        _CACHE[key] = _build(has_bias)
    nc = _CACHE[key]

    x16 = x.astype(np.float16)
    h16 = h.astype(np.float16)
    c16 = c.astype(np.float16)
    in_maps = []
    for i in range(NCORES):
        s = slice(i * BC, (i + 1) * BC)
        m = {
            "xt": np.ascontiguousarray(x16[s].T),
            "ht": np.ascontiguousarray(h16[s].T),
            "ct": np.ascontiguousarray(c16[s].T),
            "wxt": wxt,
            "wht": wht,
        }
        if has_bias:
            m["bias"] = np.ascontiguousarray(
                b.reshape(4, H).T.astype(np.float32))
        in_maps.append(m)

    res = run_bass_kernel_spmd(nc, in_maps, core_ids=list(range(NCORES)),
                               trace=trace, tmpdir=tmpdir)
    h_new = np.empty((NCORES * BC, H), dtype=np.float32)
    c_new = np.empty((NCORES * BC, H), dtype=np.float32)
    for i, r in enumerate(res.results):
        s = slice(i * BC, (i + 1) * BC)
        h_new[s] = r["hnt"].T
        c_new[s] = r["cnt"].T
    return h_new, c_new, res


def kernel(**inputs):
    h_new, c_new, _ = _run(inputs, trace=False)
    return h_new, c_new


# revision 28
# speedup vs baseline: 1.0186x; 1.0186x over previous
"""LSTMCell on 8 Trainium2 NeuronCores, data-parallel over the batch.

Full inputs: x/h_t/c_t [65536,128] f32, 8 gate weight matrices [128,128],
4 biases [128]. Returns (h_new, c_new) as [65536,128] f32 each.

Design (v13, ~59.7us; fp16 matmul path, transposed layout, no on-device
transposes; steady state is ACT(sigmoid)-bound):
  - Host transposes x/h/c per core to [128 feat, 8192 batch] fp16 and
    pre-concats weights as WxT/WhT [128 in, 512 gate-rows] fp16 in gate
    order [o, i, f, 2*g] (g prescaled by 2 for the tanh-via-sigmoid trick;
    o first so the first/last pairs can sigmoid banks i|f|2g ahead of o,
    unblocking the DVE chain ~1us earlier at the fill and tail).
    fp16 (not bf16) operands: the bf16 rounding of x/h/W through the gates
    was the dominant error term (1.2e-2); fp16 cuts it ~8x at zero cost
    (PE streams fp16 == bf16: ~216-260ns issue period per N=512 matmul).
  - Per batch group of 512 cols: 8 matmuls (weights stationary) accumulate
    gates^T into a 4-bank PSUM quad [128, 2048] = o|i|f|2g pre-acts.
  - ONE sigmoid per quad -> bf16 SBUF (ACT 16-bit-out runs ~0.9ns/elem;
    bf16 out is fastest of the 16-bit options; f32-out would be 2x faster
    on ACT but forces the DVE chain to 1x mode = net loss). Two groups
    share a sig2 tile [128, 4096] so DVE ops batch per PAIR via 3D APs
    (2-byte dtypes keep the DVE 2x mode, ~0.67ns/elem).
  - DVE per pair: gt=2s-1 [TS], ig=i*gt, fc=f*c, c'=ig+fc, h'=o*tanh(c').
    ig/fc/c'/h' and the c input are fp16: bf16 rounding of the large ig/fc
    terms dominated the error after cancellation in c'.
  - tanh(c') on ACT, delayed one pair (emitted after the next pair's
    sigmoids) so ACT never stalls on the DVE chain; batched across 2 pairs
    mid-pipeline; per-group at the tail to shorten the critical path.
  - DMA: inputs in 5 chunks/tensor (1,1,2,4,8 groups - fast pipeline fill,
    then big descriptors; 4KB-contig descriptors cap the HWDGE ring at
    ~258GB/s so later chunks use 2-8KB lines), x/h ahead of c; outputs in
    8/4/2/2-group chunks (big output DMAs fire early enough that their
    completion receipts stay off the critical tail) with per-group DMAs
    at the very end. ~9 warmup matmuls on a junk tile bridge the initial
    DMA wait so the PE's HAM activity window never resets during fill.
"""
import numpy as np
import ml_dtypes
from contextlib import ExitStack

import concourse.bass as bass
import concourse.tile as tile
from concourse import bacc, mybir
from concourse.bass_utils import run_bass_kernel_spmd

from concourse import dve_ops as _dop
from concourse.dve_spec import (
    Spec, Src0, Src1, C0, C1, C2, Bin, AluOp as DAlu, maxx, minn, sq,
    lower as _dve_lower, _has_src1,
)
from concourse.dve_uop import DveOpSpec

F32 = mybir.dt.float32
F16 = mybir.dt.float16
BF16 = mybir.dt.bfloat16
NPBF = ml_dtypes.bfloat16
AF = mybir.ActivationFunctionType
ALU = mybir.AluOpType

NCORES = 8
BC = 8192            # batch rows per core
GW = 512             # batch columns per group (one PSUM bank)
NG = BC // GW        # 16 groups
H = 128              # hidden size
# input chunks in groups: small (fast fill), then growing
ICHUNKS = [(0, 1), (1, 1), (2, 2), (4, 4), (8, 8)]
# output chunks (start group, n groups): big, medium, small tail
OCHUNKS = [(0, 8), (8, 4), (12, 2), (14, 2)]

# --- custom fused DVE ops: rational tanh(c') -------------------------------
# r = 1/(c'^2+TD) via BITWISE_NOT exponent-flip seed + one Newton step
# (fp32 internally, fp16 in/out); z = clamp((r*TC2 + TC1)*c', -1, 1).
# Joint minimax fit over the fp16 path: max |z - tanh| ~= 3.7e-3.
TR0, TR1 = -0.23549792, 2.0017324
TC1, TC2, TD = 0.12762096, 2.40399202, 2.78807243
# pairs whose tanh(c') runs on the custom DVE path (ACT keeps the rest;
# blending equalizes ACT ~4.4us/pair vs DVE ~4.2us/pair)
CUSTOM_TANH_PAIRS = (1, 2, 4, 5, 7)


def _register_dve_op(name, spec, subdim=False):
    if name in _dop._SUB_OPCODE_FOR_NAME:
        return next(op for op in _dop.OPS if op.name == name)
    row = _dop._CUSTOM_DVE_ROW_BASE + len(_dop.OPS)
    assert row < 0x20, "custom DVE row overflow"
    shas = {}
    for ver in ("v3", "v4"):
        try:
            tmp = DveOpSpec(name=name, opcode=row,
                            uops=_dve_lower(spec, ver=ver),
                            rd1_en=_has_src1(spec))
            shas[ver] = tmp.sha(ver)
        except Exception:
            pass
    op = _dop.DveOp(name, spec, subdim, shas)
    _dop.OPS.append(op)
    _dop._SUB_OPCODE_FOR_NAME[name] = row
    _dop.CUSTOM_DVE_SPECS[name] = spec
    return op


def _ref_tanh_recip(in0, in1, c0, c1, c2):
    d = (in0.astype(np.float32) ** 2 + np.float32(c0)).astype(np.float32)
    nd = (~d.view(np.int32)).view(np.float32)
    y0 = nd * np.float32(c1)
    return y0 * (np.float32(c2) - d * y0)


def _ref_tanh_zmul(in0, in1, c0, c1, c2):
    t = ((in0.astype(np.float32) * np.float32(c0) + np.float32(c1))
         * in1.astype(np.float32))
    return np.maximum(np.minimum(t, np.float32(c2)), -np.float32(c2))


_d = sq(Src0) + C0
_nd = Bin(DAlu.BITWISE_NOT, _d, _d)
_y0 = _nd * C1
TANH_RECIP = _register_dve_op(
    "LSTM_TANH_RECIP", Spec(body=_y0 * (C2 - _d * _y0),
                            reference=_ref_tanh_recip))
_t = (Src0 * C0 + C1) * Src1
TANH_ZMUL = _register_dve_op(
    "LSTM_TANH_ZMUL", Spec(body=maxx(minn(_t, C2), -C2),
                           reference=_ref_tanh_zmul))

_CACHE = {}


def _build(has_bias: bool):
    nc = bacc.Bacc("TRN2", target_bir_lowering=False, debug=False)
    xt = nc.dram_tensor("xt", [H, BC], F16, kind="ExternalInput").ap()
    ht = nc.dram_tensor("ht", [H, BC], F16, kind="ExternalInput").ap()
    ct = nc.dram_tensor("ct", [H, BC], F16, kind="ExternalInput").ap()
    wxt = nc.dram_tensor("wxt", [H, 4 * H], F16, kind="ExternalInput").ap()
    wht = nc.dram_tensor("wht", [H, 4 * H], F16, kind="ExternalInput").ap()
    if has_bias:
        bias = nc.dram_tensor("bias", [H, 4], F32, kind="ExternalInput").ap()
    hnt = nc.dram_tensor("hnt", [H, BC], F16, kind="ExternalOutput").ap()
    cnt = nc.dram_tensor("cnt", [H, BC], F16, kind="ExternalOutput").ap()


    with tile.TileContext(nc) as tc:
        with ExitStack() as ctx:
            const = ctx.enter_context(tc.tile_pool(name="const", bufs=1))
            ina = ctx.enter_context(tc.tile_pool(name="ina", bufs=1))
            qp = ctx.enter_context(tc.tile_pool(name="qp", bufs=2, space="PSUM"))
            tp = ctx.enter_context(tc.tile_pool(name="tp", bufs=3))
            sp = ctx.enter_context(tc.tile_pool(name="sp", bufs=6))
            op = ctx.enter_context(tc.tile_pool(name="op", bufs=3))

            # Input tiles in chunks per tensor: small chunk first for fast
            # pipeline fill, then medium/large for DMA efficiency.
            xts, hts, cts = [], [], []
            for ci, (cs, cw) in enumerate(ICHUNKS):
                for lst, nm in ((xts, "x"), (hts, "h"), (cts, "c")):
                    lst.append(ina.tile([H, cw * GW], F16,
                                        name=f"{nm}{ci}"))
            def cbounds(ci):
                cs, cw = ICHUNKS[ci]
                return cs * GW, (cs + cw) * GW
            # gpsimd queue first: warmup/dummy memsets (so the PE warmups
            # are not blocked behind any queue preamble), then the weights.
            # NOTE: never issue DMA from the scalar queue - HWDGE on the
            # Activation engine evicts its ACT table (~1.3us reload).
            junk = const.tile([H, GW], F16)
            nc.gpsimd.memset(junk[:], 0.0)
            dummy = const.tile([H, 8], F32)
            nc.gpsimd.memset(dummy[:], 0.0)
            wx_sb = const.tile([H, 4 * H], F16)
            nc.gpsimd.dma_start(wx_sb[:], wxt)
            wh_sb = const.tile([H, 4 * H], F16)
            nc.gpsimd.dma_start(wh_sb[:], wht)
            if has_bias:
                b_sb = const.tile([H, 4], F32)
                nc.gpsimd.dma_start(b_sb[:], bias)
            # x/h chunks 0-3 gate matmuls -> issue them first on the sync
            # queue; c trails (consumed later by the chain).  One queue =>
            # transfer order matches consumption order.
            nchunk = len(ICHUNKS)
            order = ([("xh", ci) for ci in range(nchunk - 1)] +
                     [("c", ci) for ci in range(3)] +
                     [("xh", nchunk - 1)] +
                     [("c", ci) for ci in range(3, nchunk)])
            for kind, ci in order:
                c0, c1 = cbounds(ci)
                if kind == "xh":
                    nc.sync.dma_start(xts[ci][:], xt[:, c0:c1])
                    nc.sync.dma_start(hts[ci][:], ht[:, c0:c1])
                else:
                    nc.sync.dma_start(cts[ci][:], ct[:, c0:c1])

            # ACT table preload (sigmoid set includes tanh) overlaps DMA fill
            dummy2 = const.tile([H, 8], F32)
            nc.scalar.activation(dummy2[:], dummy[:], AF.Sigmoid)

            def in_slice(tiles, g, w):
                c0 = g * GW
                for ci, (cs, cw) in enumerate(ICHUNKS):
                    if c0 + w <= (cs + cw) * GW:
                        return tiles[ci][:, c0 - cs * GW:c0 - cs * GW + w]
                raise AssertionError("slice straddles input chunks")

            # HAM warmup on the junk tile while DMAs stream; >=8 needed:
            # HAM un-throttle wants ~3.4us of SUSTAINED PE activity, else
            # the PE runs at half clock for the WHOLE kernel
            warm = qp.tile([H, 2048], F32, name="warm", tag="quad")
            for _ in range(9):
                nc.tensor.matmul(warm[:, 0:GW], junk[:, 0:H], junk[:],
                                 start=True, stop=True)

            NP = NG // 2  # pairs
            sig2s = {}

            # pair -> (chunk_start_group, chunk_width, local_offset, is_last)
            pair_chunk = {}
            for cs, cw in OCHUNKS:
                for g in range(cs, cs + cw, 2):
                    pair_chunk[g // 2] = (cs, cw * GW, (g - cs) * GW,
                                          g + 2 == cs + cw)

            def emit_tanh_h2(Pa):
                """tanh + h' for pairs (Pa, Pa+1) in one ACT pass."""
                Pb = Pa + 1
                cs, cw, lo_a, _ = pair_chunk[Pa]
                cnb, hnb = cn_hn[Pa]
                tcp = tp.tile([H, 2048], BF16, name=f"tc{Pa}", tag="tc")
                nc.scalar.activation(tcp[:], cnb[:, lo_a:lo_a + 4 * GW],
                                     AF.Tanh)
                for j, P in enumerate((Pa, Pb)):
                    lo = pair_chunk[P][2]
                    last = pair_chunk[P][3]
                    sig2 = sig2s.pop(P)
                    o3 = sig2[:].rearrange("p (t x) -> p t x",
                                           t=2)[:, :, 0:512]
                    h3 = hnb[:, lo:lo + 2 * GW].rearrange(
                        "p (t x) -> p t x", t=2)
                    t3 = tcp[:, j * 1024:(j + 1) * 1024].rearrange(
                        "p (t x) -> p t x", t=2)
                    nc.vector.tensor_mul(h3, o3, t3)
                    if last:
                        nc.sync.dma_start(hnt[:, cs * GW:cs * GW + cw],
                                          hnb[:])

            def emit_tanh_h(P):
                """tanh + h' + (maybe) hn DMA for pair P (c' already done)."""
                cs, cw, lo, last = pair_chunk[P]
                cnb, hnb = cn_hn[P]
                tcp = tp.tile([H, 1024], BF16, name=f"tc{P}", tag="tc")
                nc.scalar.activation(tcp[:], cnb[:, lo:lo + 2 * GW], AF.Tanh)
                sig2 = sig2s.pop(P)
                o3 = sig2[:].rearrange("p (t x) -> p t x", t=2)[:, :, 0:512]
                h3 = hnb[:, lo:lo + 2 * GW].rearrange("p (t x) -> p t x", t=2)
                t3 = tcp[:].rearrange("p (t x) -> p t x", t=2)
                nc.vector.tensor_mul(h3, o3, t3)
                if last:
                    nc.sync.dma_start(hnt[:, cs * GW:cs * GW + cw], hnb[:])

            def emit_custom_h(P):
                """custom r/z (rational tanh) + h' + (maybe) hn DMA for
                pair P (c' already done).  Deferred one pair so this ~3us
                of DVE work covers the pool fc latency of the next pair."""
                cs, cw, lo, last = pair_chunk[P]
                cnb, hnb = cn_hn[P]
                cpr = cnb[:, lo:lo + 2 * GW]
                rt = tp.tile([H, 2 * GW], F16, name=f"r{P}", tag="rt")
                nc.vector._custom_dve(TANH_RECIP, out=rt[:], in0=cpr,
                                      s0=TD, s1=TR0, imm2=TR1)
                zt = tp.tile([H, 2 * GW], F16, name=f"z{P}", tag="zt")
                nc.vector._custom_dve(TANH_ZMUL, out=zt[:], in0=rt[:],
                                      in1=cpr, s0=TC2, s1=TC1, imm2=1.0)
                sig2c = sig2s.pop(P)
                o3 = sig2c[:].rearrange("p (t x) -> p t x",
                                        t=2)[:, :, 0:512]
                h3 = hnb[:, lo:lo + 2 * GW].rearrange(
                    "p (t x) -> p t x", t=2)
                z3 = zt[:].rearrange("p (t x) -> p t x", t=2)
                nc.vector.tensor_mul(h3, o3, z3)
                if last:
                    nc.sync.dma_start(hnt[:, cs * GW:cs * GW + cw], hnb[:])

            cn_hn = {}
            cn_buf = hn_buf = None
            for P in range(NP):
                g0 = 2 * P
                cs, cw, lo, last = pair_chunk[P]
                if lo == 0:
                    cn_buf = op.tile([H, cw], F16, name=f"cn{g0}", tag="cn")
                    hn_buf = op.tile([H, cw], F16, name=f"hn{g0}", tag="hn")
                cn_hn[P] = (cn_buf, hn_buf)
                sig2 = sp.tile([H, 4096], BF16, name=f"s{P}", tag="sig")
                sig2s[P] = sig2

                def emit_dve(g_first, ng, tag_sfx):
                    """c'-chain for ng groups starting at g_first (pair P).
                    ig/fc/c' are fp16: bf16 rounding of the large ig/fc
                    terms would dominate the error after cancellation."""
                    w = ng * GW
                    gg = g_first - g0

                    def sl(bank):
                        s = sig2[:].rearrange("p (t x) -> p t x", t=2)
                        s = s[:, gg:gg + ng, bank * GW:(bank + 1) * GW]
                        return s

                    def r3(ap2d):
                        return ap2d.rearrange("p (t x) -> p t x", t=ng)

                    c3 = r3(in_slice(cts, g_first, w))
                    gt = tp.tile([H, w], BF16, name=f"gt{tag_sfx}", tag="gt")
                    nc.vector.tensor_scalar(r3(gt[:]), sl(0 + 3), 2.0, 1.0,
                                            ALU.mult, ALU.subtract)
                    ig = tp.tile([H, w], F16, name=f"ig{tag_sfx}", tag="ig")
                    nc.vector.tensor_mul(r3(ig[:]), sl(1), r3(gt[:]))
                    fc = tp.tile([H, w], F16, name=f"fc{tag_sfx}", tag="fc")
                    nc.gpsimd.tensor_mul(r3(fc[:]), sl(2), c3)
                    lg = lo + gg * GW
                    nc.vector.tensor_add(cn_buf[:, lg:lg + w], ig[:], fc[:])
                    if last and gg + ng == 2:
                        nc.sync.dma_start(
                            cnt[:, cs * GW:cs * GW + cw], cn_buf[:])

                lastP = P == NP - 1
                for gg in range(2):
                    g = g0 + gg
                    xs = in_slice(xts, g, GW)
                    hs = in_slice(hts, g, GW)
                    split = (lastP or P == 0) and not has_bias
                    quad = qp.tile([H, 2048], F32, name=f"q{g}", tag="quad")
                    so = sig2[:, gg * 2048:(gg + 1) * 2048]
                    for k in ([1, 2, 3, 0] if split else range(4)):
                        nc.tensor.matmul(quad[:, k * GW:(k + 1) * GW],
                                         wx_sb[:, k * H:(k + 1) * H], xs,
                                         start=True, stop=False)
                        nc.tensor.matmul(quad[:, k * GW:(k + 1) * GW],
                                         wh_sb[:, k * H:(k + 1) * H], hs,
                                         start=False, stop=True)
                    if has_bias:
                        for k in range(4):
                            nc.scalar.activation(
                                so[:, k * GW:(k + 1) * GW],
                                quad[:, k * GW:(k + 1) * GW],
                                AF.Sigmoid, bias=b_sb[:, k:k + 1])
                    elif split:
                        # i/f/s banks first: unblocks the DVE chain; the
                        # o bank (only needed by h') trails
                        nc.scalar.activation(so[:, GW:], quad[:, GW:],
                                             AF.Sigmoid)
                        nc.scalar.activation(so[:, 0:GW], quad[:, 0:GW],
                                             AF.Sigmoid)
                    else:
                        nc.scalar.activation(so, quad[:], AF.Sigmoid)
                    if lastP or P == 0:
                        # per-group chain: shortens tail (last pair) and
                        # avoids straddling input chunks (first pair)
                        emit_dve(g, 1, f"p{P}g{gg}")
                    if gg == 1 and P >= 1:
                        if (P - 1) in CUSTOM_TANH_PAIRS:
                            emit_custom_h(P - 1)
                        else:
                            emit_tanh_h(P - 1)

                if not (lastP or P == 0):
                    emit_dve(g0, 2, f"p{P}")

            # last pair: per-group custom r/z/h'/hn to shorten the tail
            P = NP - 1
            cs, cw, lo, _ = pair_chunk[P]
            cnb, hnb = cn_hn[P]
            sig2 = sig2s.pop(P)
            for gg in range(2):
                lg = lo + gg * GW
                cps = cnb[:, lg:lg + GW]
                rt = tp.tile([H, GW], F16, name=f"rz{gg}", tag="rt")
                nc.vector._custom_dve(TANH_RECIP, out=rt[:], in0=cps,
                                      s0=TD, s1=TR0, imm2=TR1)
                zt = tp.tile([H, GW], F16, name=f"zz{gg}", tag="zt")
                nc.vector._custom_dve(TANH_ZMUL, out=zt[:], in0=rt[:],
                                      in1=cps, s0=TC2, s1=TC1, imm2=1.0)
                o2 = sig2[:, gg * 2048:gg * 2048 + 512]
                nc.vector.tensor_mul(hnb[:, lg:lg + GW], o2, zt[:])
                gcol = (cs + gg * (cw // GW - 1)) * GW
                nc.sync.dma_start(hnt[:, gcol:gcol + GW],
                                  hnb[:, lg:lg + GW])
    nc.compile()
    return nc


def _run(inputs, trace=False, tmpdir=None):
    x = np.asarray(inputs["x"], dtype=np.float32)
    h = np.asarray(inputs["h_t"], dtype=np.float32)
    c = np.asarray(inputs["c_t"], dtype=np.float32)
    # gate order [i, f, o, g]; W_g/b_g scaled by 2 for the tanh-via-sigmoid
    wx = np.concatenate([inputs["W_io"], inputs["W_ii"], inputs["W_if"],
                         2.0 * np.asarray(inputs["W_ig"])], axis=0)
    wh = np.concatenate([inputs["W_ho"], inputs["W_hi"], inputs["W_hf"],
                         2.0 * np.asarray(inputs["W_hg"])], axis=0)
    b = np.concatenate([inputs["b_o"], inputs["b_i"], inputs["b_f"],
                        2.0 * np.asarray(inputs["b_g"])], axis=0)
    wxt = np.ascontiguousarray(wx.T).astype(np.float16)
    wht = np.ascontiguousarray(wh.T).astype(np.float16)
    has_bias = bool(np.any(b))

    key = has_bias
    if key not in _CACHE:
        _CACHE[key] = _build(has_bias)
    nc = _CACHE[key]

    x16 = x.astype(np.float16)
    h16 = h.astype(np.float16)
    c16 = c.astype(np.float16)
    in_maps = []
    for i in range(NCORES):
        s = slice(i * BC, (i + 1) * BC)
        m = {
            "xt": np.ascontiguousarray(x16[s].T),
            "ht": np.ascontiguousarray(h16[s].T),
            "ct": np.ascontiguousarray(c16[s].T),
            "wxt": wxt,
            "wht": wht,
        }
        if has_bias:
            m["bias"] = np.ascontiguousarray(
                b.reshape(4, H).T.astype(np.float32))
        in_maps.append(m)

    res = run_bass_kernel_spmd(nc, in_maps, core_ids=list(range(NCORES)),
                               trace=trace, tmpdir=tmpdir)
    h_new = np.empty((NCORES * BC, H), dtype=np.float32)
    c_new = np.empty((NCORES * BC, H), dtype=np.float32)
    for i, r in enumerate(res.results):
        s = slice(i * BC, (i + 1) * BC)
        h_new[s] = r["hnt"].T
        c_new[s] = r["cnt"].T
    return h_new, c_new, res


def kernel(**inputs):
    h_new, c_new, _ = _run(inputs, trace=False)
    return h_new, c_new

